# revision 33
# baseline (speedup 1.0000x reference)
"""Trainium2 Bass kernel for DifferentialAttention (B=2, S=2048, DIM=2048).

Sharding: 8 cores = 2 batches x 4 head-groups (4 heads each). Per core:
  - QKV projection + RoPE on device, differential attention for 4 heads,
  - row-parallel wo partial product; host sums the 4 partials per batch.

v3 design (cost-model driven):
  * Projections (Q/K/V) and wo run in fp8e4 DoubleRow perf mode (0.5
    cycles/row, 2 k-tiles per call -> 4x bf16 matmul throughput). Accuracy
    is held at bf16 level with a hi/lo split: x ~ (e4m3(16x)+e4m3(res))/16
    against w_hi, plus an x/2 copy against the 32x-boosted w residual.
    All three terms accumulate in one f32 psum group (scale 64).
  * Scores are produced TRANSPOSED ([k,q]: lhsT=KT tile, rhs=QT tile,
    8 k-tile matmuls into a [128,8,128] psum tile) so the exp on the Act
    engine writes etT directly in the layout PV needs -- the former
    [q,k]->[k,q] XBAR DMA transposes (120us of serialized DMA) vanish.
  * Softmax denominators come for free from PV: V carries an appended
    ones column (rhs width 129), so ppv[:,128] = sum_k p. No accum_out
    on the exps -> cheaper Act instructions.
  * wo: atT is split post-transpose into (e4m3(8at), residual, /32 copy)
    and multiplied against wo_hi/wo_lo fp8 in DoubleRow (6 calls/nch).
  * u' = lam*(d1/d2)*pv2 - pv1 fused into one scalar_tensor_tensor with
    a per-partition scalar; rsqrt via Quake bit-trick + 2 Newton steps.
  * The 64x fp8 scale cancels algebraically: exp scale = SCALE/4096,
    rsqrt arg scaled so ya = rsqrt(true)/8 (t8 = 8*at for the fp8 split),
    host divides the summed partials by 128.
  * A phase is serial PE-dense fp8 (no BC weave); its x-DMA-gated head
    interleaves K/V region groups across all six free psum banks so the
    PE chews chunks as they land. All weights hoisted ahead of the
    x-half-1 stream. BC steps balance PE ~9.4us / Act ~8.8us / DVE ~8us;
    pipeline skew: scores/exp at step s, PV+norm at s-1, wo+out at s-3
    (the extra step gives the u'->rsqrt->t8->XBAR-transpose->at-split
    chain a full step of slack; splits run on the idle GPSIMD engine).
  * PSUM discipline: accumulation groups sharing a 2KB bank must be
    temporally contiguous (a start marks the whole bank pending-zero).

Per-core layouts (partition dim first):
  QT [128,4,S]: q heads; rows = [branch j | quadrant q | E(16) O(16)],
     row j*64+q*32+c*16+i <-> head-dim 2*(16q+i)+c. Values 64x scaled.
  KT [128,2,S]: same for the 2 kv heads.
  Vn [128,16,2,129]: v natural [s, dv] layout + ones column, s-tile major.
  etT [128,8,8,128]: [k-in-tile, hj, ktile, q] per query tile.
"""

import math
import numpy as np
import ml_dtypes
import concourse.bass as bass
import concourse.tile as tile
from concourse import bacc, mybir
from concourse.bass_utils import run_bass_kernel_spmd
from contextlib import ExitStack

F32 = mybir.dt.float32
BF16 = mybir.dt.bfloat16
FP8 = mybir.dt.float8e4
AF = mybir.ActivationFunctionType
ALU = mybir.AluOpType
DR = mybir.MatmulPerfMode.DoubleRow

DIM = 2048
S = 2048
B = 2
HD = 64          # rope head dim
EPS = 1e-5
SCALE = HD ** -0.5
ESC = SCALE / 4096.0   # exp scale on the 64x-scaled scores psum
NCORES = 8
NQT = S // 128   # 16 query tiles

TRACE = False
LAST_RESULTS = None
DBG = None       # set to a dict to get QT/KT/Vn/etT/u/t8 debug outputs

# stream_shuffle mask: swap 16-row halves within each 32-partition quadrant
_SWAP16 = list(range(16, 32)) + list(range(0, 16))

F8NP = ml_dtypes.float8_e4m3
BFNP = ml_dtypes.bfloat16


# ---------------------------------------------------------------- device program

def build_program(lam: float):
    nc = bacc.Bacc("TRN2", target_bir_lowering=False, debug=False,
                   num_devices=NCORES)
    io = {}
    for name, shape, d in [
        ("xhi", [DIM, S], FP8), ("xlo", [DIM, S], FP8), ("xh2", [DIM, S], FP8),
        ("wq_hi", [DIM, 512], FP8), ("wq_lo", [DIM, 512], FP8),
        ("wk_hi", [DIM, 256], FP8), ("wk_lo", [DIM, 256], FP8),
        ("wv_hi", [DIM, 256], FP8), ("wv_lo", [DIM, 256], FP8),
        ("wo_hi", [512, DIM], FP8), ("wo_lo", [512, DIM], FP8),
        ("cs128", [128, S], BF16), ("sn128", [128, S], BF16),
    ]:
        io[name] = nc.dram_tensor(name, shape, d, kind="ExternalInput").ap()
    out = nc.dram_tensor("out", [S, DIM], BF16, kind="ExternalOutput").ap()

    if DBG is not None:
        for name, shape, d in [
            ("dQT", [128, 4, S], BF16), ("dKT", [128, 2, S], BF16),
            ("dVn", [128, 16, 2, 129], BF16),
            ("detT", [128, 8, 8, 128], BF16),
            ("du", [128, 4, 128], BF16), ("dt8", [128, 4, 128], BF16),
            ("dt8T", [128, 4, 128], BF16),
        ]:
            DBG[name] = nc.dram_tensor(name, shape, d,
                                       kind="ExternalOutput").ap()

    with tile.TileContext(nc) as tc:
        _body(tc, io, out, lam)
    nc.compile()
    return nc


def _body(tc, io, out, lam):
    nc = tc.nc
    with ExitStack() as top:
        persist = top.enter_context(tc.tile_pool(name="persist", bufs=1))
        QT = persist.tile([128, 4, S], BF16)
        KT = persist.tile([128, 2, S], BF16)
        Vn = persist.tile([128, 16, 2, 129], BF16)
        # ones column for the free softmax denominators (bf16 1.0 = 0x3F80)
        nc.gpsimd.memset(Vn[:, :, :, 128:129].bitcast(mybir.dt.uint16), 0x3F80)

        _stage_a(tc, io, QT, KT, Vn)
        if DBG is not None:
            nc.sync.dma_start(DBG["dQT"], QT[:])
            nc.sync.dma_start(DBG["dKT"], KT[:])
            nc.sync.dma_start(DBG["dVn"], Vn[:])
        _stage_bc(tc, io, out, QT, KT, Vn, lam)


# ------------------------------------------------------------------- A stage

def _rope(tc, a, pq, dest_ap, ssl):
    """c0 = evac(pq); dest = c0*cs + shuffle(c0*sn)."""
    nc = tc.nc
    W = 1024
    c0 = a["tmp"].tile([128, W], BF16, tag="c0", name="c0")
    nc.scalar.activation(c0[:], pq[:], AF.Copy, bias=0.0, scale=1.0)
    t2 = a["tmp"].tile([128, W], BF16, tag="t2", name="t2")
    t2s = a["tmp"].tile([128, W], BF16, tag="t2s", name="t2s")
    nc.gpsimd.tensor_mul(t2[:], c0[:], a["sn"][:, ssl])
    nc.vector.stream_shuffle(t2s[:], t2[:], _SWAP16)
    nc.vector.tensor_mul(c0[:], c0[:], a["cs"][:, ssl])
    nc.vector.tensor_add(dest_ap, c0[:], t2s[:])


def _mm3(nc, pq_ap, terms, nchunks, nsz):
    """24 DoubleRow matmuls per 256-col chunk: 3 (x,w)-terms x 8 k-pairs.

    Each psum region's accumulation group must be temporally contiguous:
    a `start` on ANY region marks its whole 2KB psum bank pending-zero,
    so an interleaved sibling group would restart mid-accumulation."""
    last_t = len(terms) - 1
    for n in range(nchunks):
        nsl = slice(n * nsz, (n + 1) * nsz)
        for t in range(8):
            for ti, (xt, wt, wsl) in enumerate(terms):
                nc.tensor.matmul(
                    pq_ap[:, nsl],
                    lhsT=wt[:, 2 * t:2 * t + 2, wsl],
                    rhs=xt[:, 2 * t:2 * t + 2, nsl],
                    start=(ti == 0 and t == 0),
                    stop=(ti == last_t and t == 7),
                    perf_mode=DR)


def _stage_a(tc, io, QT, KT, Vn):
    nc = tc.nc
    with ExitStack() as actx:
        a = {}
        a["xp"] = actx.enter_context(tc.tile_pool(name="xp", bufs=2))
        a["trig"] = actx.enter_context(tc.tile_pool(name="trig", bufs=1))
        a["tmp"] = actx.enter_context(tc.tile_pool(name="ropetmp", bufs=2))
        a["wp"] = actx.enter_context(tc.tile_pool(name="wqk", bufs=2))
        a["wvp"] = actx.enter_context(tc.tile_pool(name="wvp", bufs=1))
        with ExitStack() as pctx:
            psqk = pctx.enter_context(tc.tile_pool(name="ps_qk", bufs=2,
                                                   space="PSUM"))
            psv = pctx.enter_context(tc.tile_pool(name="ps_v", bufs=2,
                                                  space="PSUM"))
            psv2 = pctx.enter_context(tc.tile_pool(name="ps_v2", bufs=2,
                                                   space="PSUM"))
            x3 = {n: io[n].rearrange("(a p) s -> p a s", p=128)
                  for n in ("xhi", "xlo", "xh2")}
            wq3 = {n: io["wq_" + n].rearrange("(a p) c -> p a c", p=128)
                   for n in ("hi", "lo")}
            wk3 = {n: io["wk_" + n].rearrange("(a p) c -> p a c", p=128)
                   for n in ("hi", "lo")}
            wv3 = {n: io["wv_" + n].rearrange("(a p) c -> p a c", p=128)
                   for n in ("hi", "lo")}

            # x half 0: the three copies' k-chunks interleaved so the
            # first (t-ordered) matmul triples start after ~3 chunk DMAs.
            # All projection weights are hoisted up front so no mid-phase
            # load ever queues behind the 19us x-half streams.
            xh0 = {}
            for n in ("xhi", "xlo", "xh2"):
                xh0[n] = a["xp"].tile([128, 16, 1024], FP8, tag=n,
                                      name=f"{n}0")
            wk_t = {}
            for n in ("hi", "lo"):
                wk_t[n] = a["wp"].tile([128, 16, 256], FP8, name=f"wk{n}")
            for n in ("xhi", "xlo", "xh2"):
                nc.sync.dma_start(xh0[n][:, 0:2, :], x3[n][:, 0:2, 0:1024])
            for n in ("hi", "lo"):
                nc.sync.dma_start(wk_t[n][:], wk3[n][:])
            for c0, c1 in ((2, 4), (4, 8), (8, 12), (12, 16)):
                for n in ("xhi", "xlo", "xh2"):
                    nc.sync.dma_start(xh0[n][:, c0:c1, :],
                                      x3[n][:, c0:c1, 0:1024])
                if c0 == 2:
                    cs = a["trig"].tile([128, S], BF16)
                    sn = a["trig"].tile([128, S], BF16)
                    nc.sync.dma_start(cs[:, 0:1024], io["cs128"][:, 0:1024])
                    nc.sync.dma_start(sn[:, 0:1024], io["sn128"][:, 0:1024])
            # weights after the x half-0 gate but ahead of the x half-1 queue
            wv_t = {}
            for n in ("hi", "lo"):
                wv_t[n] = a["wvp"].tile([128, 16, 256], FP8, name=f"wv{n}")
                nc.sync.dma_start(wv_t[n][:], wv3[n][:])
            wq_t = {}
            for n in ("hi", "lo"):
                wq_t[n] = a["wp"].tile([128, 16, 512], FP8, name=f"wq{n}")
                nc.sync.dma_start(wq_t[n][:], wq3[n][:])
            nc.sync.dma_start(cs[:, 1024:2048], io["cs128"][:, 1024:2048])
            nc.sync.dma_start(sn[:, 1024:2048], io["sn128"][:, 1024:2048])
            a["cs"], a["sn"] = cs, sn

            for sq in (0, 1):
                ssl = slice(sq * 1024, sq * 1024 + 1024)
                if sq == 0:
                    xh = xh0
                else:
                    xh = {}
                    for n in ("xhi", "xlo", "xh2"):
                        xh[n] = a["xp"].tile([128, 16, 1024], FP8, tag=n,
                                             name=f"{n}1")
                        nc.sync.dma_start(xh[n][:], x3[n][:, :, ssl])
                # K: 2 kv tiles
                def emit_k(kt_i):
                    ws = slice(kt_i * 128, (kt_i + 1) * 128)
                    pq = psqk.tile([128, 1024], F32, tag="psqk", name="pqk")
                    terms = [(xh["xhi"], wk_t["hi"], ws),
                             (xh["xlo"], wk_t["hi"], ws),
                             (xh["xh2"], wk_t["lo"], ws)]
                    _mm3(nc, pq, terms, 4, 256)
                    _rope(tc, a, pq, KT[:, kt_i, ssl], ssl)
                # V: 8 s-tiles (lhsT = x chunk, rhs = wv)
                def emit_v(st):
                    pv = psv.tile([128, 256], F32, tag="psv", name="psv")
                    for t in range(8):
                        for ti, (xn, wn) in enumerate(
                                [("xhi", "hi"), ("xlo", "hi"), ("xh2", "lo")]):
                            nc.tensor.matmul(
                                pv[:],
                                lhsT=xh[xn][:, 2 * t:2 * t + 2,
                                            st * 128:(st + 1) * 128],
                                rhs=wv_t[wn][:, 2 * t:2 * t + 2, :],
                                start=(t == 0 and ti == 0),
                                stop=(t == 7 and ti == 2),
                                perf_mode=DR)
                    for kv in range(2):
                        nc.vector.tensor_copy(
                            Vn[:, sq * 8 + st, kv, 0:128],
                            pv[:, kv * 128:(kv + 1) * 128])
                # Q: 4 head tiles
                def emit_q(ct):
                    ws = slice(ct * 128, (ct + 1) * 128)
                    pq = psqk.tile([128, 1024], F32, tag="psqk", name="pqq")
                    terms = [(xh["xhi"], wq_t["hi"], ws),
                             (xh["xlo"], wq_t["hi"], ws),
                             (xh["xh2"], wq_t["lo"], ws)]
                    _mm3(nc, pq, terms, 4, 256)
                    _rope(tc, a, pq, QT[:, ct, ssl], ssl)

                def v_evac(pv, st):
                    for kv in range(2):
                        nc.vector.tensor_copy(
                            Vn[:, sq * 8 + st, kv, 0:128],
                            pv[:, kv * 128:(kv + 1) * 128])

                if sq == 0:
                    # startup is x-DMA-gated: interleave region groups across
                    # all six psum banks (K kt0/kt1 nch-parity regions + two
                    # V s-tiles) so the PE chews each x chunk as it lands.
                    # Interleaved groups always sit in different psum banks.
                    pqk0 = psqk.tile([128, 1024], F32, tag="psqk", name="pqk0")
                    pqk1 = psqk.tile([128, 1024], F32, tag="psqk", name="pqk1")
                    pqk = (pqk0, pqk1)
                    kterms = [(xh["xhi"], wk_t["hi"]), (xh["xlo"], wk_t["hi"]),
                              (xh["xh2"], wk_t["lo"])]
                    vterms = [("xhi", "hi"), ("xlo", "hi"), ("xh2", "lo")]
                    pvs = {}
                    for par in (0, 1):
                        sts = (0, 1, 2, 3) if par == 0 else (4, 5, 6, 7)
                        for st in sts:
                            # full-bank tiles, two pools: interleaved groups
                            # must not share a psum bank (per-bank zeroing)
                            vp = psv if st % 2 == 0 else psv2
                            pvs[st] = vp.tile([128, 512], F32, tag="psv",
                                              name=f"pv{st}")
                        for t in range(8):
                            for ti in range(3):
                                xt, wt = kterms[ti]
                                for kt_i in (0, 1):
                                    ws = slice(kt_i * 128, (kt_i + 1) * 128)
                                    for n in (par, par + 2):
                                        nsl = slice(n * 256, (n + 1) * 256)
                                        nc.tensor.matmul(
                                            pqk[kt_i][:, nsl],
                                            lhsT=wt[:, 2 * t:2 * t + 2, ws],
                                            rhs=xt[:, 2 * t:2 * t + 2, nsl],
                                            start=(t == 0 and ti == 0),
                                            stop=(t == 7 and ti == 2),
                                            perf_mode=DR)
                                xn, wn = vterms[ti]
                                for st in sts:
                                    nc.tensor.matmul(
                                        pvs[st][:, 0:256],
                                        lhsT=xh[xn][:, 2 * t:2 * t + 2,
                                                    st * 128:(st + 1) * 128],
                                        rhs=wv_t[wn][:, 2 * t:2 * t + 2, :],
                                        start=(t == 0 and ti == 0),
                                        stop=(t == 7 and ti == 2),
                                        perf_mode=DR)
                        for st in sts:
                            v_evac(pvs[st][:, 0:256], st)
                    _rope(tc, a, pqk0, KT[:, 0, ssl], ssl)
                    _rope(tc, a, pqk1, KT[:, 1, ssl], ssl)
                    for ct in range(4):
                        emit_q(ct)
                else:
                    # BC step 0 needs QT/KT complete; Vn only from step 1 on.
                    emit_k(0)
                    emit_k(1)
                    for ct in range(4):
                        emit_q(ct)
                    for st in range(8):
                        emit_v(st)


# ------------------------------------------------------------------ BC stage

def _stage_bc(tc, io, out, QT, KT, Vn, lam):
    """Attention + norm + wo, pipelined over 16 query tiles, skew 1/2.

    step s: scoresT+exp(qt=s) | pv+norm(qt=s-1) | at-split+wo+out(qt=s-2).
    """
    nc = tc.nc
    H_ORDER = (0, 2, 1, 3)
    with ExitStack() as ctx:
        etp = [ctx.enter_context(tc.tile_pool(name="etpA", bufs=1)),
               ctx.enter_context(tc.tile_pool(name="etpB", bufs=1))]
        wotp = ctx.enter_context(tc.tile_pool(name="wotp", bufs=1))
        colp = ctx.enter_context(tc.tile_pool(name="colp", bufs=3))
        up = ctx.enter_context(tc.tile_pool(name="up", bufs=2))
        t8p = ctx.enter_context(tc.tile_pool(name="t8p", bufs=2))
        t8Tp = ctx.enter_context(tc.tile_pool(name="t8Tp", bufs=3))
        atxp = ctx.enter_context(tc.tile_pool(name="atxp", bufs=3))
        outwp = ctx.enter_context(tc.tile_pool(name="outwp", bufs=2))
        magp = ctx.enter_context(tc.tile_pool(name="magic", bufs=1))
        magicT = magp.tile([128, 4], mybir.dt.uint32, name="magicT")
        nc.gpsimd.memset(magicT[:], 0x5F3759DF)

        wot = {}
        for n in ("hi", "lo"):
            wot[n] = wotp.tile([128, 4, S], FP8, name=f"wot{n}")
            nc.sync.dma_start(wot[n][:], io["wo_" + n].rearrange(
                "(a p) c -> p a c", p=128))

        pss = ctx.enter_context(tc.tile_pool(name="pss", bufs=2,
                                             space="PSUM"))
        ppvp = ctx.enter_context(tc.tile_pool(name="ppv", bufs=1,
                                              space="PSUM"))
        psop = ctx.enter_context(tc.tile_pool(name="pso", bufs=1,
                                              space="PSUM"))

        etT_t = [None] * NQT
        t8T_t = [None] * NQT
        atx_t = [None] * NQT

        for step in range(NQT + 3):
            fr = step if step < NQT else None
            mid = step - 1 if 1 <= step <= NQT else None
            bk = step - 3 if step >= 3 else None
            spl = step - 2 if 2 <= step < NQT + 2 else None

            if fr is not None:
                etT = etp[fr % 2].tile([128, 8, 8, 128], BF16, tag="et",
                                       name=f"et{fr}")
                etT_t[fr] = etT
            if mid is not None:
                ppvA = ppvp.tile([128, 512], F32, tag="ppvA")
                ppvB = ppvp.tile([128, 512], F32, tag="ppvB")
                ppvC = ppvp.tile([128, 512], F32, tag="ppvC")

                def slot(h, j):
                    flat = h * 2 + j
                    t = (ppvA, ppvB, ppvC)[flat // 3]
                    off = (flat % 3) * 129
                    return t[:, off:off + 129]
                u = up.tile([128, 4, 128], BF16, tag="u")
                v2 = up.tile([128, 128], BF16, tag="v2")
                usq = up.tile([128, 128], BF16, tag="usq")
                dc = colp.tile([128, 4], F32, tag="dc")
                rec = colp.tile([128, 4], F32, tag="rec")
                rrl = colp.tile([128, 4], F32, tag="rrl")
                msum = colp.tile([128, 4], F32, tag="msum")
                tcol = colp.tile([128, 4], F32, tag="tcol")
                ya = colp.tile([128, 4], F32, tag="ya")
                aa = colp.tile([128, 4], F32, tag="aa")
                shu = colp.tile([128, 4], mybir.dt.uint32, tag="shu")
            if spl is not None:
                # split t8T[spl] a full step before its wo stage consumes it
                t8Tb = t8T_t[spl]
                athi = atxp.tile([128, 4, 128], FP8, tag="athi")
                atlo = atxp.tile([128, 4, 128], FP8, tag="atlo")
                ath4 = atxp.tile([128, 4, 128], FP8, tag="ath4")
                if step > NQT:
                    # drain: DVE is idle and the splits gate the tail
                    nc.vector.tensor_copy(athi[:], t8Tb[:])
                    nc.gpsimd.tensor_sub(atlo[:], t8Tb[:], athi[:])
                    nc.vector.tensor_scalar(ath4[:], athi[:], 0.03125, None,
                                            op0=ALU.mult)
                else:
                    nc.gpsimd.tensor_copy(athi[:], t8Tb[:])
                    nc.gpsimd.tensor_sub(atlo[:], t8Tb[:], athi[:])
                    nc.gpsimd.tensor_scalar(ath4[:], athi[:], 0.03125, None,
                                            op0=ALU.mult)
                atx_t[spl] = (athi, atlo, ath4)
            if bk is not None:
                athi, atlo, ath4 = atx_t[bk]
                outw = outwp.tile([128, 2048], BF16, tag="outw")
                bkst = {}

                def emit_bk(beat):
                    nch = beat
                    if nch % 2 == 0:
                        # drain steps: alternate with a borrowed ppv bank so
                        # the wo psum is double-buffered once scores are gone
                        if step > NQT and nch % 4 == 2:
                            bkst["pso"] = ppvp.tile([128, 512], F32,
                                                    tag="ppvA",
                                                    name=f"psoB{nch}")
                        else:
                            bkst["pso"] = psop.tile([128, 512], F32,
                                                    tag="pso",
                                                    name=f"psoA{nch}")
                    pso = bkst["pso"]
                    reg = pso[:, (nch % 2) * 256:(nch % 2) * 256 + 256]
                    csl = slice(nch * 256, (nch + 1) * 256)
                    i = 0
                    for ax, wn in ((athi, "hi"), (atlo, "hi"), (ath4, "lo")):
                        for r in range(2):
                            nc.tensor.matmul(
                                reg,
                                lhsT=ax[:, 2 * r:2 * r + 2, :],
                                rhs=wot[wn][:, 2 * r:2 * r + 2, csl],
                                start=(i == 0), stop=(i == 5),
                                perf_mode=DR)
                            i += 1
                    if nch % 2 == 1:
                        osl = slice((nch - 1) * 256, (nch + 1) * 256)
                        if step >= NQT and nch % 4 == 1:
                            # no exps in drain steps -> Act is free
                            nc.scalar.activation(outw[:, osl], pso[:],
                                                 AF.Copy, bias=0.0, scale=1.0)
                        else:
                            nc.vector.tensor_copy(outw[:, osl], pso[:])
                        if step == NQT + 2:
                            # tail: fire each finished quarter immediately
                            nc.sync.dma_start(
                                out[bk * 128:(bk + 1) * 128, osl],
                                outw[:, osl])

                if step >= NQT:
                    # drain: wo work is ready now, pv beats are exp-gated;
                    # keep the in-order PE queue from head-blocking on pv
                    for beat in range(8):
                        emit_bk(beat)

            for beat in range(8):
                # frontend: transposed scores + exp for one (h, j)
                if fr is not None:
                    hj = beat
                    j, h = divmod(hj, 4)
                    kvl, rho = h // 2, h % 2
                    jsl = slice(j * 64, j * 64 + 64)
                    qsl = slice(fr * 128, fr * 128 + 128)
                    psc = pss.tile([128, 8, 128], F32, tag="sc")
                    for kt in range(8):
                        kof = rho * 1024 + kt * 128
                        nc.tensor.matmul(
                            psc[:, kt, :],
                            lhsT=KT[jsl, kvl, kof:kof + 128],
                            rhs=QT[jsl, h, qsl],
                            start=True, stop=True)
                    nc.scalar.activation(etT[:, hj, :, :], psc[:], AF.Exp,
                                         bias=0.0, scale=float(ESC))
                # middle: pv for one (h, j); after j=1 the u'/norm column ops
                if mid is not None:
                    h = H_ORDER[beat // 2]
                    j = beat % 2
                    hj = j * 4 + h
                    kvl, rho = h // 2, h % 2
                    etm = etT_t[mid]
                    pdst = slot(h, j)
                    for kt in range(8):
                        nc.tensor.matmul(
                            pdst,
                            lhsT=etm[:, hj, kt, :],
                            rhs=Vn[:, rho * 8 + kt, kvl, :],
                            start=(kt == 0), stop=(kt == 7))
                    if j == 1:
                        s0, s1 = slot(h, 0), slot(h, 1)
                        nc.vector.reciprocal(rec[:, h:h + 1], s1[:, 128:129])
                        nc.vector.scalar_tensor_tensor(
                            rrl[:, h:h + 1], s0[:, 128:129], float(lam),
                            rec[:, h:h + 1], op0=ALU.mult, op1=ALU.mult)
                        nc.vector.tensor_scalar_mul(
                            v2[:], s1[:, 0:128], rrl[:, h:h + 1])
                        nc.vector.tensor_sub(u[:, h, :], v2[:], s0[:, 0:128])
                        nc.vector.scalar_tensor_tensor(
                            usq[:], u[:, h, :], 1.0, u[:, h, :],
                            op0=ALU.mult, op1=ALU.mult,
                            accum_out=msum[:, h:h + 1])
                        nc.vector.tensor_copy(dc[:, h:h + 1], s0[:, 128:129])
                # backend: one 256-col chunk of the fp8 wo matmul
                if bk is not None and step < NQT:
                    emit_bk(beat)


            if mid is not None:
                # ya = rsqrt(msum/8192 + 64*eps*d1^2) = rsqrt(true)/8
                # Quake bit-trick + two Newton steps on the DVE
                nc.vector.scalar_tensor_tensor(
                    tcol[:], dc[:], float(64.0 * EPS), dc[:],
                    op0=ALU.mult, op1=ALU.mult)
                nc.vector.scalar_tensor_tensor(
                    tcol[:], msum[:], float(1.0 / 8192.0), tcol[:],
                    op0=ALU.mult, op1=ALU.add)
                nc.vector.tensor_scalar(shu[:], tcol[:].bitcast(
                    mybir.dt.uint32), 1, None, op0=ALU.arith_shift_right)
                nc.vector.tensor_sub(ya[:].bitcast(mybir.dt.uint32),
                                     magicT[:], shu[:])
                for _ in range(2):
                    nc.vector.tensor_mul(aa[:], ya[:], ya[:])
                    nc.vector.tensor_mul(aa[:], aa[:], tcol[:])
                    nc.vector.tensor_scalar(aa[:], aa[:], -0.5, 1.5,
                                            op0=ALU.mult, op1=ALU.add)
                    nc.vector.tensor_mul(ya[:], ya[:], aa[:])
                t8 = t8p.tile([128, 4, 128], BF16, tag="t8")
                for h in range(4):
                    nc.vector.tensor_scalar_mul(t8[:, h, :], u[:, h, :],
                                                ya[:, h:h + 1])
                t8T = t8Tp.tile([128, 4, 128], BF16, tag="t8T")
                t8T_t[mid] = t8T
                nc.sync.dma_start_transpose(t8T[:], t8[:])
                if DBG is not None and mid == 0:
                    nc.sync.dma_start(DBG["detT"], etT_t[0][:])
                    nc.sync.dma_start(DBG["du"], u[:])
                    nc.sync.dma_start(DBG["dt8"], t8[:])
                    nc.sync.dma_start(DBG["dt8T"], t8T[:])

            if bk is not None and step < NQT + 2:
                nc.gpsimd.dma_start(out[bk * 128:(bk + 1) * 128, :], outw[:])


# ---------------------------------------------------------------- host side

# row (q*32 + c*16 + i) within a branch <-> head-dim 2*(16q+i)+c
_PERM64 = np.empty(64, np.int64)
for _q in range(2):
    for _c in range(2):
        for _i in range(16):
            _PERM64[_q * 32 + _c * 16 + _i] = 2 * (16 * _q + _i) + _c


def _hilo_w(w, boost):
    w4 = (boost * w).astype(np.float32)
    hi = w4.astype(F8NP)
    lo = (32.0 * (w4 - hi.astype(np.float32))).astype(F8NP)
    return hi, lo


def make_core_inputs(core, x, wq, wk, wv, wo, subln_w, lambda_init,
                     freqs_cos, freqs_sin):
    b, g = divmod(core, 4)
    qcols = np.empty(512, np.int64)
    for hl in range(4):
        for j in range(2):
            qcols[hl * 128 + j * 64:hl * 128 + j * 64 + 64] = \
                ((4 * g + hl) * 2 + j) * 64 + _PERM64
    kcols = np.empty(256, np.int64)
    for kvl in range(2):
        for j in range(2):
            kcols[kvl * 128 + j * 64:kvl * 128 + j * 64 + 64] = \
                ((2 * g + kvl) * 2 + j) * 64 + _PERM64
    vcols = np.arange(256) + 2 * g * 128

    cosT = np.ascontiguousarray(freqs_cos.T.astype(np.float32))  # [32, S]
    sinT = np.ascontiguousarray(freqs_sin.T.astype(np.float32))
    cs64 = np.concatenate([cosT[0:16], cosT[0:16],
                           cosT[16:32], cosT[16:32]], axis=0)
    sn64 = np.concatenate([sinT[0:16], -sinT[0:16],
                           sinT[16:32], -sinT[16:32]], axis=0)

    xT = np.ascontiguousarray(x[b].T.astype(np.float32))
    x16 = 16.0 * xT
    xhi = x16.astype(F8NP)
    xlo = (x16 - xhi.astype(np.float32)).astype(F8NP)
    xh2 = (0.5 * xT).astype(F8NP)

    wq_hi, wq_lo = _hilo_w(wq[:, qcols].astype(np.float32), 4.0)
    wk_hi, wk_lo = _hilo_w(wk[:, kcols].astype(np.float32), 4.0)
    wv_hi, wv_lo = _hilo_w(wv[:, vcols].astype(np.float32), 4.0)

    # wo rows carry subln*(1-lambda_init) and the global sign flip (u' = -u)
    wo_eff = wo[512 * g: 512 * g + 512, :].astype(np.float32).copy()
    wo_eff *= -np.tile(subln_w.astype(np.float32)
                       * (1.0 - np.float32(np.asarray(lambda_init)[0])),
                       4)[:, None]
    wo_hi, wo_lo = _hilo_w(wo_eff, 16.0)

    return {
        "xhi": xhi, "xlo": xlo, "xh2": xh2,
        "wq_hi": wq_hi, "wq_lo": wq_lo,
        "wk_hi": wk_hi, "wk_lo": wk_lo,
        "wv_hi": wv_hi, "wv_lo": wv_lo,
        "wo_hi": wo_hi, "wo_lo": wo_lo,
        "cs128": np.tile(cs64, (2, 1)).astype(BFNP),
        "sn128": np.tile(sn64, (2, 1)).astype(BFNP),
    }


def compute_lambda(lambda_q1, lambda_k1, lambda_q2, lambda_k2, lambda_init):
    l1 = np.exp(np.sum(np.float32(lambda_q1) * np.float32(lambda_k1),
                       dtype=np.float32))
    l2 = np.exp(np.sum(np.float32(lambda_q2) * np.float32(lambda_k2),
                       dtype=np.float32))
    return float(l1 - l2 + np.float32(np.asarray(lambda_init)[0]))


def kernel(x, wq, wk, wv, wo, lambda_q1, lambda_k1, lambda_q2, lambda_k2,
           lambda_init, subln_w, freqs_cos, freqs_sin):
    global LAST_RESULTS
    x = np.asarray(x); wq = np.asarray(wq); wk = np.asarray(wk)
    wv = np.asarray(wv); wo = np.asarray(wo)
    lam = compute_lambda(lambda_q1, lambda_k1, lambda_q2, lambda_k2,
                         lambda_init)

    nc = build_program(lam)
    in_maps = [make_core_inputs(c, x, wq, wk, wv, wo,
                                np.asarray(subln_w), np.asarray(lambda_init),
                                np.asarray(freqs_cos), np.asarray(freqs_sin))
               for c in range(NCORES)]
    res = run_bass_kernel_spmd(nc, in_maps, list(range(NCORES)), trace=TRACE)
    LAST_RESULTS = res
    outs = [res.results[c]["out"] for c in range(NCORES)]
    full = np.empty((B, S, DIM), np.float32)
    for b in range(B):
        full[b] = (outs[4 * b].astype(np.float32)
                   + outs[4 * b + 1].astype(np.float32)
                   + outs[4 * b + 2].astype(np.float32)
                   + outs[4 * b + 3].astype(np.float32)) * (1.0 / 128.0)
    return full
